# revision 23
# baseline (speedup 1.0000x reference)
"""Causal self-attention (RoPE + QK-RMSNorm, GQA 16q/8kv) Trainium2 Bass kernel.

Sharding: 8 cores = 2 batch x 4 tensor-parallel. Core c handles batch b=c//4 and
q-heads [4*tp, 4*tp+4), kv-heads [2*tp, 2*tp+2) where tp=c%4. Each core returns a
partial (T, C) output = O_heads @ wo[rows of its heads]; host sums the 4 partials
per batch (the "all-reduce after c_proj").

v4: single interleaved pipeline (QKV chunk c -> attention span c -> c_proj of
span c). Partition reductions (RMS ms, softmax denominators) and [1,512]
broadcasts all run on the PE (ones-matmuls, 213ns each) -- GpSimd is avoided
entirely (its ops carry ~1.5us sequencer/semaphore latency). V is computed
directly in natural [t,d] layout (x-block stationary), so no PE transposes.
rstd/recip chains stay on ACT (Ln/Exp); a post-compile pass forces the single
combined ln+exp+copy activation table so there is no table thrash. The single
ot PSUM bank is drained to SBUF by an ACT copy immediately after the AV
accumulation so the next head's matmuls are not blocked by the normalize chain.
"""
import sys
import math

sys.path.insert(0, "/opt/trn_rl_repo")

import numpy as np
import ml_dtypes
import concourse.bacc as bacc
import concourse.mybir as mybir
import concourse.tile as tile
from concourse.bass_utils import run_bass_kernel_spmd

P = 128
T = 2048
C = 2048
KO = C // P          # 16 contraction tiles
D = 128              # head dim
NQ = 4               # q heads per core
NK = 2               # kv heads per core
NF = NQ + NK         # 6 rope/rms feature blocks (4 q + 2 k)
FQ = NQ * D          # 512
FK = NK * D          # 256
TCH = 512            # T-chunk = q-span
NCHUNK = T // TCH    # 4
SPAN = TCH
KB = T // P          # 16 key blocks
SCALE = 1.0 / math.sqrt(D)
EPS = 1.1920929e-07

f32 = mybir.dt.float32
f32r = mybir.dt.float32r
bf16 = mybir.dt.bfloat16

AF = mybir.ActivationFunctionType

# index of 'natural_log_exp_and_others' in act_info.json act_func_sets
ACT_TABLE_LN_EXP = 6


def _force_single_act_table(nc):
    """Replace the compiler's thrashing ACT table loads (alternating
    natural_log / exp_and_others, 1.28us each) with a single load of the
    combined ln+exp+copy table per block."""
    n_kept = 0
    for fn in nc.m.functions:
        for b in fn.blocks:
            newinsts = []
            seen = False
            for inst in b.instructions:
                if isinstance(inst, mybir.InstLoadActFuncSet):
                    if seen:
                        continue
                    inst.act_func_set_id = ACT_TABLE_LN_EXP
                    seen = True
                    n_kept += 1
                newinsts.append(inst)
            b.instructions[:] = newinsts
    return n_kept


def build():
    nc = bacc.Bacc("TRN2", target_bir_lowering=False)
    xT = nc.dram_tensor("xT", (C, T), bf16, kind="ExternalInput")
    wq = nc.dram_tensor("wq", (C, FQ), bf16, kind="ExternalInput")
    wk = nc.dram_tensor("wk", (C, FK), bf16, kind="ExternalInput")
    wv = nc.dram_tensor("wv", (C, FK), bf16, kind="ExternalInput")
    wo = nc.dram_tensor("wo", (FQ, C), bf16, kind="ExternalInput")
    cc = nc.dram_tensor("cc", (P, T), bf16, kind="ExternalInput")    # [cos; cos]
    ss = nc.dram_tensor("ss", (P, T), bf16, kind="ExternalInput")    # [sin; -sin]
    maskT = nc.dram_tensor("maskT", (P, 4, SPAN), bf16, kind="ExternalInput")
    y = nc.dram_tensor("y", (T, C), bf16, kind="ExternalOutput")

    xT_r = xT.rearrange("(ko p) t -> p ko t", p=P)
    wq_r = wq.rearrange("(ko p) f -> p ko f", p=P)
    wk_r = wk.rearrange("(ko p) f -> p ko f", p=P)
    wv_r = wv.rearrange("(ko p) f -> p ko f", p=P)
    wo_r = wo.rearrange("(ko p) n -> p ko n", p=P)

    with tile.TileContext(nc) as tc:
        with (
            tc.tile_pool(name="persist", bufs=1) as persist,
            tc.tile_pool(name="xp", bufs=2) as xpool,
            tc.tile_pool(name="rp", bufs=2) as rpool,
            tc.tile_pool(name="ptp", bufs=3) as ppool,
            tc.tile_pool(name="ap", bufs=4) as apool,
            tc.tile_pool(name="yp", bufs=2) as ypool,
            tc.tile_pool(name="mm", bufs=2, space="PSUM") as mm,
            tc.tile_pool(name="stat", bufs=1, space="PSUM") as statp,
            tc.tile_pool(name="st", bufs=2, space="PSUM") as spool,
            tc.tile_pool(name="ot", bufs=1, space="PSUM") as pot,
        ):
            qk_rt = persist.tile([P, NF, T], bf16, tag="qk_rt")   # roped+normed qT/kT
            v_sb = persist.tile([P, KB, FK], bf16, tag="v_sb")    # V natural [t-part, kb, feat]
            cc_sb = persist.tile([P, T], bf16, tag="cc_sb")
            ss_sb = persist.tile([P, T], bf16, tag="ss_sb")
            ot_sb = persist.tile([P, NQ, T], bf16, tag="ot_sb")
            mask_sb = persist.tile([P, 4, SPAN], bf16, tag="mask_sb")
            wo_sb = persist.tile([P, NQ, C], bf16, tag="wo_sb")
            wq_sb = persist.tile([P, KO, FQ], bf16, tag="wq_sb")
            wk_sb = persist.tile([P, KO, FK], bf16, tag="wk_sb")
            wv_sb = persist.tile([P, KO, FK], bf16, tag="wv_sb")
            eps_sb = persist.tile([P, 1], f32, tag="eps_sb")
            ones_c = persist.tile([P, 1], bf16, tag="ones_c")
            ones_r = persist.tile([1, P], f32r, tag="ones_r")
            ones_rf = persist.tile([1, P], f32, tag="ones_rf")
            nc.vector.memset(eps_sb[:], EPS)
            nc.vector.memset(ones_c[:], 1.0)
            nc.vector.memset(ones_rf[:], 1.0)
            nc.vector.tensor_copy(ones_r[:], ones_rf[:])
            # only what chunk 0's q/k needs goes first; wv/mask/wo are
            # emitted lazily below so the first matmul isn't stuck behind
            # 8MB of weight DMAs
            for ko in range(KO):
                nc.sync.dma_start(wq_sb[:, ko, :], wq_r[:, ko, :])
                nc.sync.dma_start(wk_sb[:, ko, :], wk_r[:, ko, :])
            nc.sync.dma_start(cc_sb[:], cc[:, :])
            nc.sync.dma_start(ss_sb[:], ss[:, :])

            for tch in range(NCHUNK):
                t0 = tch * TCH
                # ---------------- QKV chunk tch ----------------
                xt = xpool.tile([P, KO, TCH], bf16, tag="xt")
                for ko in range(KO):
                    nc.sync.dma_start(xt[:, ko, :], xT_r[:, ko, t0 : t0 + TCH])
                # q/k blocks: rope + rms norm into qk_rt
                for fb in range(NF):
                    if fb < NQ:
                        w_ap = wq_sb[:, :, fb * D : (fb + 1) * D]
                    else:
                        w_ap = wk_sb[:, :, (fb - NQ) * D : (fb - NQ + 1) * D]
                    pqk = mm.tile([P, TCH], f32, tag="mm")
                    for ko in range(KO):
                        nc.tensor.matmul(
                            pqk[:], w_ap[:, ko], xt[:, ko, :],
                            start=(ko == 0), stop=(ko == KO - 1),
                        )
                    raw = rpool.tile([P, TCH], bf16, tag="raw")
                    nc.vector.tensor_copy(raw[:], pqk[:])
                    swp = rpool.tile([P, TCH], bf16, tag="swp")
                    nc.sync.dma_start(swp[0:64, :], raw[64:128, :])
                    nc.sync.dma_start(swp[64:128, :], raw[0:64, :])
                    tmpa = rpool.tile([P, TCH], bf16, tag="tmpa")
                    tmpb = rpool.tile([P, TCH], bf16, tag="tmpb")
                    nc.vector.tensor_mul(tmpa[:], raw[:], cc_sb[:, t0 : t0 + TCH])
                    nc.vector.tensor_mul(tmpb[:], swp[:], ss_sb[:, t0 : t0 + TCH])
                    segt = rpool.tile([P, TCH], bf16, tag="segt")
                    nc.vector.tensor_add(segt[:], tmpa[:], tmpb[:])
                    # rms: ms = ones^T @ segt^2 (PE), rstd = exp(-.5 ln(ms/D+eps))
                    # on [1,512] (ACT), broadcast via PE ones-row matmul
                    sq = rpool.tile([P, TCH], bf16, tag="sq")
                    nc.vector.tensor_mul(sq[:], segt[:], segt[:])
                    ms = statp.tile([1, TCH], f32, tag="stat")
                    nc.tensor.matmul(ms[:], ones_c[:], sq[:], start=True, stop=True)
                    lnm = rpool.tile([1, TCH], f32, tag="lnm")
                    nc.scalar.activation(
                        lnm[:], ms[:], AF.Ln, bias=eps_sb[0:1, :], scale=1.0 / D
                    )
                    rstd = rpool.tile([1, TCH], f32r, tag="rstd")
                    nc.scalar.activation(rstd[:], lnm[:], AF.Exp, scale=-0.5)
                    bc_ps = pot.tile([P, TCH], f32, tag="ot")
                    nc.tensor.matmul(bc_ps[:], ones_r[:], rstd[:], start=True, stop=True)
                    nc.vector.tensor_mul(qk_rt[:, fb, t0 : t0 + TCH], segt[:], bc_ps[:])
                if tch == 0:
                    nc.sync.dma_start(wv_sb[:], wv_r)
                    nc.sync.dma_start(mask_sb[:], maskT[:, :, :])
                    nc.sync.dma_start(wo_sb[:], wo_r)
                # V directly in natural [t, d] layout: x-block stationary
                for tb in range(TCH // P):
                    vn = mm.tile([P, TCH], f32, tag="mm")
                    for ko in range(KO):
                        nc.tensor.matmul(
                            vn[:, 0:FK],
                            xt[:, ko, tb * P : (tb + 1) * P],
                            wv_sb[:, ko, :],
                            start=(ko == 0), stop=(ko == KO - 1),
                        )
                    nc.vector.tensor_copy(v_sb[:, tch * 4 + tb, :], vn[:, 0:FK])

                # ---------------- attention span s = tch ----------------
                s = tch
                q0 = s * SPAN
                nkp = 2 * s + 2          # kb pairs
                deferred = []            # per-head normalize chains, emitted at span end
                for h in range(NQ):
                    j = h // 2
                    q_ap = qk_rt[:, h, q0 : q0 + SPAN]
                    ot_ps = pot.tile([P, SPAN], f32, tag="ot")
                    sum_ps = statp.tile([1, SPAN], f32, tag="stat")

                    def scores(p):
                        st2 = spool.tile([P, 2, SPAN], f32, tag="st")
                        for i in range(2):
                            nc.tensor.matmul(
                                st2[:, i, :],
                                qk_rt[:, NQ + j, (2 * p + i) * P : (2 * p + i + 1) * P],
                                q_ap,
                                start=True, stop=True,
                            )
                        return st2

                    st_cur = scores(0)
                    for p in range(nkp):
                        st_nxt = scores(p + 1) if p + 1 < nkp else None
                        pt2 = ppool.tile([P, 2, SPAN], bf16, tag="pt")
                        nc.scalar.activation(pt2[:], st_cur[:], AF.Exp, scale=SCALE)
                        if p >= 2 * s:  # diagonal pairs need the causal mask
                            moff = 2 * (p - 2 * s)
                            nc.vector.tensor_mul(
                                pt2[:], pt2[:], mask_sb[:, moff : moff + 2, :]
                            )
                        for i in range(2):
                            kb = 2 * p + i
                            nc.tensor.matmul(
                                ot_ps[:],
                                v_sb[:, kb, j * D : (j + 1) * D],
                                pt2[:, i, :],
                                start=(kb == 0), stop=(kb == 2 * nkp - 1),
                                skip_group_check=True,
                            )
                            nc.tensor.matmul(
                                sum_ps[:],
                                ones_c[:],
                                pt2[:, i, :],
                                start=(kb == 0), stop=(kb == 2 * nkp - 1),
                                skip_group_check=True,
                            )
                        st_cur = st_nxt
                    # drain ot to SBUF fast on DVE (frees the single ot bank
                    # without using ACT); Ln releases the stat slot right
                    # away; the rest of the normalize chain is deferred to
                    # span end so it never delays the next head's exps
                    otc = apool.tile([P, SPAN], bf16, tag="otc")
                    nc.vector.tensor_copy(otc[:], ot_ps[:])
                    lns = apool.tile([1, SPAN], f32, tag="lns")
                    nc.scalar.activation(lns[:], sum_ps[:], AF.Ln)

                    def normalize(h=h, otc=otc, lns=lns):
                        rec = apool.tile([1, SPAN], f32r, tag="rec")
                        nc.scalar.activation(rec[:], lns[:], AF.Exp, scale=-1.0)
                        bc_ps = mm.tile([P, TCH], f32, tag="mm")
                        nc.tensor.matmul(
                            bc_ps[:], ones_r[:], rec[:], start=True, stop=True
                        )
                        nc.vector.tensor_mul(
                            ot_sb[:, h, q0 : q0 + SPAN], otc[:], bc_ps[:]
                        )

                    deferred.append(normalize)
                for fn in deferred:
                    fn()

                # ---------------- output projection for this span ----------------
                for tb in range(4 * s, 4 * s + 4):
                    for nch in range(C // 512):
                        yps = mm.tile([P, TCH], f32, tag="mm")
                        for h in range(NQ):
                            nc.tensor.matmul(
                                yps[:],
                                ot_sb[:, h, tb * P : (tb + 1) * P],
                                wo_sb[:, h, nch * 512 : (nch + 1) * 512],
                                start=(h == 0), stop=(h == NQ - 1),
                            )
                        ysb = ypool.tile([P, 512], bf16, tag="ysb")
                        nc.vector.tensor_copy(ysb[:], yps[:])
                        nc.sync.dma_start(
                            y[tb * P : (tb + 1) * P, nch * 512 : (nch + 1) * 512],
                            ysb[:],
                        )
    nc.compile()
    n = _force_single_act_table(nc)
    print(f"act-table surgery: kept {n} table loads")
    return nc


_NC_CACHE = None


def _get_nc():
    global _NC_CACHE
    if _NC_CACHE is None:
        _NC_CACHE = build()
    return _NC_CACHE


def _host_inputs(x, cos, sin, wq, wk, wv, wo):
    """Build the 8 per-core input maps."""
    bft = ml_dtypes.bfloat16
    cosT = np.ascontiguousarray(cos[0, :, 0, :].T).astype(np.float32)  # (64, T)
    sinT = np.ascontiguousarray(sin[0, :, 0, :].T).astype(np.float32)
    cc = np.concatenate([cosT, cosT], axis=0).astype(bft)     # (128, T)
    ss = np.concatenate([sinT, -sinT], axis=0).astype(bft)
    # maskT[r][k, q] = 1 if q >= 128*r + k  (within a 512-q span, k-block offset r)
    qidx = np.arange(SPAN)[None, None, :]
    kidx = np.arange(P)[:, None, None]
    ridx = np.arange(4)[None, :, None]
    maskT = (qidx >= P * ridx + kidx).astype(bft)  # (128, 4, 512)

    xTs = [np.ascontiguousarray(x[b].T).astype(bft) for b in range(2)]
    wq16 = wq.astype(bft)
    wk16 = wk.astype(bft)
    wv16 = wv.astype(bft)
    wo16 = wo.astype(bft)
    in_maps = []
    for c in range(8):
        b, tp = divmod(c, 4)
        in_maps.append(
            {
                "xT": xTs[b],
                "wq": np.ascontiguousarray(wq16[:, tp * FQ : (tp + 1) * FQ]),
                "wk": np.ascontiguousarray(wk16[:, tp * FK : (tp + 1) * FK]),
                "wv": np.ascontiguousarray(wv16[:, tp * FK : (tp + 1) * FK]),
                "wo": np.ascontiguousarray(wo16[tp * FQ : (tp + 1) * FQ, :]),
                "cc": cc,
                "ss": ss,
                "maskT": maskT,
            }
        )
    return in_maps


def kernel(x, cos, sin, wq, wk, wv, wo, trace=False):
    x = np.asarray(x, dtype=np.float32)
    cos = np.asarray(cos, dtype=np.float32)
    sin = np.asarray(sin, dtype=np.float32)
    wq = np.asarray(wq, dtype=np.float32)
    wk = np.asarray(wk, dtype=np.float32)
    wv = np.asarray(wv, dtype=np.float32)
    wo = np.asarray(wo, dtype=np.float32)

    nc = _get_nc()
    in_maps = _host_inputs(x, cos, sin, wq, wk, wv, wo)
    res = run_bass_kernel_spmd(nc, in_maps, core_ids=list(range(8)), trace=trace)
    out = np.zeros((2, T, C), dtype=np.float32)
    for c in range(8):
        b = c // 4
        out[b] += np.asarray(res.results[c]["y"], dtype=np.float32)
    if trace:
        return out, res
    return out


# revision 26
# speedup vs baseline: 1.0014x; 1.0014x over previous
"""Causal self-attention (RoPE + QK-RMSNorm, GQA 16q/8kv) Trainium2 Bass kernel.

Sharding: 8 cores = 2 batch x 4 tensor-parallel. Core c handles batch b=c//4 and
q-heads [4*tp, 4*tp+4), kv-heads [2*tp, 2*tp+2) where tp=c%4. Each core returns a
partial (T, C) output = O_heads @ wo[rows of its heads]; host sums the 4 partials
per batch (the "all-reduce after c_proj").

v4: single interleaved pipeline (QKV chunk c -> attention span c -> c_proj of
span c). Partition reductions (RMS ms, softmax denominators) and [1,512]
broadcasts all run on the PE (ones-matmuls, 213ns each) -- GpSimd is avoided
entirely (its ops carry ~1.5us sequencer/semaphore latency). V is computed
directly in natural [t,d] layout (x-block stationary), so no PE transposes.
rstd/recip chains stay on ACT (Ln/Exp); a post-compile pass forces the single
combined ln+exp+copy activation table so there is no table thrash. The single
ot PSUM bank is drained to SBUF by an ACT copy immediately after the AV
accumulation so the next head's matmuls are not blocked by the normalize chain.
"""
import sys
import math

sys.path.insert(0, "/opt/trn_rl_repo")

import numpy as np
import ml_dtypes
import concourse.bacc as bacc
import concourse.mybir as mybir
import concourse.tile as tile
from concourse.bass_utils import run_bass_kernel_spmd

P = 128
T = 2048
C = 2048
KO = C // P          # 16 contraction tiles
D = 128              # head dim
NQ = 4               # q heads per core
NK = 2               # kv heads per core
NF = NQ + NK         # 6 rope/rms feature blocks (4 q + 2 k)
FQ = NQ * D          # 512
FK = NK * D          # 256
TCH = 512            # T-chunk = q-span
NCHUNK = T // TCH    # 4
SPAN = TCH
KB = T // P          # 16 key blocks
SCALE = 1.0 / math.sqrt(D)
EPS = 1.1920929e-07

f32 = mybir.dt.float32
f32r = mybir.dt.float32r
bf16 = mybir.dt.bfloat16

AF = mybir.ActivationFunctionType

# index of 'natural_log_exp_and_others' in act_info.json act_func_sets
ACT_TABLE_LN_EXP = 6


def _force_single_act_table(nc):
    """Replace the compiler's thrashing ACT table loads (alternating
    natural_log / exp_and_others, 1.28us each) with a single load of the
    combined ln+exp+copy table per block."""
    n_kept = 0
    for fn in nc.m.functions:
        for b in fn.blocks:
            newinsts = []
            seen = False
            for inst in b.instructions:
                if isinstance(inst, mybir.InstLoadActFuncSet):
                    if seen:
                        continue
                    inst.act_func_set_id = ACT_TABLE_LN_EXP
                    seen = True
                    n_kept += 1
                newinsts.append(inst)
            b.instructions[:] = newinsts
    return n_kept


def build():
    nc = bacc.Bacc("TRN2", target_bir_lowering=False)
    xT = nc.dram_tensor("xT", (C, T), bf16, kind="ExternalInput")
    wq = nc.dram_tensor("wq", (C, FQ), bf16, kind="ExternalInput")
    wk = nc.dram_tensor("wk", (C, FK), bf16, kind="ExternalInput")
    wv = nc.dram_tensor("wv", (C, FK), bf16, kind="ExternalInput")
    wo = nc.dram_tensor("wo", (FQ, C), bf16, kind="ExternalInput")
    cc = nc.dram_tensor("cc", (P, T), bf16, kind="ExternalInput")    # [cos; cos]
    ss = nc.dram_tensor("ss", (P, T), bf16, kind="ExternalInput")    # [sin; -sin]
    maskT = nc.dram_tensor("maskT", (P, 4, SPAN), bf16, kind="ExternalInput")
    y = nc.dram_tensor("y", (T, C), bf16, kind="ExternalOutput")

    xT_r = xT.rearrange("(ko p) t -> p ko t", p=P)
    wq_r = wq.rearrange("(ko p) f -> p ko f", p=P)
    wk_r = wk.rearrange("(ko p) f -> p ko f", p=P)
    wv_r = wv.rearrange("(ko p) f -> p ko f", p=P)
    wo_r = wo.rearrange("(ko p) n -> p ko n", p=P)

    with tile.TileContext(nc) as tc:
        with (
            tc.tile_pool(name="persist", bufs=1) as persist,
            tc.tile_pool(name="xp", bufs=2) as xpool,
            tc.tile_pool(name="rp", bufs=2) as rpool,
            tc.tile_pool(name="ptp", bufs=3) as ppool,
            tc.tile_pool(name="ap", bufs=4) as apool,
            tc.tile_pool(name="yp", bufs=2) as ypool,
            tc.tile_pool(name="mm", bufs=2, space="PSUM") as mm,
            tc.tile_pool(name="stat", bufs=1, space="PSUM") as statp,
            tc.tile_pool(name="st", bufs=2, space="PSUM") as spool,
            tc.tile_pool(name="ot", bufs=1, space="PSUM") as pot,
        ):
            qk_rt = persist.tile([P, NF, T], bf16, tag="qk_rt")   # roped+normed qT/kT
            v_sb = persist.tile([P, KB, FK], bf16, tag="v_sb")    # V natural [t-part, kb, feat]
            cc_sb = persist.tile([P, T], bf16, tag="cc_sb")
            ss_sb = persist.tile([P, T], bf16, tag="ss_sb")
            ot_sb = persist.tile([P, NQ, T], bf16, tag="ot_sb")
            mask_sb = persist.tile([P, 4, SPAN], bf16, tag="mask_sb")
            wo_sb = persist.tile([P, NQ, C], bf16, tag="wo_sb")
            wq_sb = persist.tile([P, KO, FQ], bf16, tag="wq_sb")
            wk_sb = persist.tile([P, KO, FK], bf16, tag="wk_sb")
            wv_sb = persist.tile([P, KO, FK], bf16, tag="wv_sb")
            eps_sb = persist.tile([P, 1], f32, tag="eps_sb")
            ones_c = persist.tile([P, 1], bf16, tag="ones_c")
            ones_r = persist.tile([1, P], f32r, tag="ones_r")
            ones_rf = persist.tile([1, P], f32, tag="ones_rf")
            nc.vector.memset(eps_sb[:], EPS)
            nc.vector.memset(ones_c[:], 1.0)
            nc.vector.memset(ones_rf[:], 1.0)
            nc.vector.tensor_copy(ones_r[:], ones_rf[:])
            # only what chunk 0's q/k needs goes first; wv/mask/wo are
            # emitted lazily below so the first matmul isn't stuck behind
            # 8MB of weight DMAs
            nc.sync.dma_start(cc_sb[:], cc[:, :])
            nc.sync.dma_start(ss_sb[:], ss[:, :])

            for tch in range(NCHUNK):
                t0 = tch * TCH
                # ---------------- QKV chunk tch ----------------
                xt = xpool.tile([P, KO, TCH], bf16, tag="xt")
                for ko in range(KO):
                    nc.sync.dma_start(xt[:, ko, :], xT_r[:, ko, t0 : t0 + TCH])
                    if tch == 0:
                        # interleave weight slices with x slices so the first
                        # matmuls aren't stuck behind whole-weight DMAs
                        nc.sync.dma_start(wq_sb[:, ko, :], wq_r[:, ko, :])
                        nc.sync.dma_start(wk_sb[:, ko, :], wk_r[:, ko, :])
                # q/k blocks: rope + rms norm into qk_rt
                for fb in range(NF):
                    if fb < NQ:
                        w_ap = wq_sb[:, :, fb * D : (fb + 1) * D]
                    else:
                        w_ap = wk_sb[:, :, (fb - NQ) * D : (fb - NQ + 1) * D]
                    pqk = mm.tile([P, TCH], f32, tag="mm")
                    for ko in range(KO):
                        nc.tensor.matmul(
                            pqk[:], w_ap[:, ko], xt[:, ko, :],
                            start=(ko == 0), stop=(ko == KO - 1),
                        )
                    raw = rpool.tile([P, TCH], bf16, tag="raw")
                    nc.vector.tensor_copy(raw[:], pqk[:])
                    swp = rpool.tile([P, TCH], bf16, tag="swp")
                    nc.sync.dma_start(swp[0:64, :], raw[64:128, :])
                    nc.sync.dma_start(swp[64:128, :], raw[0:64, :])
                    tmpa = rpool.tile([P, TCH], bf16, tag="tmpa")
                    tmpb = rpool.tile([P, TCH], bf16, tag="tmpb")
                    nc.vector.tensor_mul(tmpa[:], raw[:], cc_sb[:, t0 : t0 + TCH])
                    nc.vector.tensor_mul(tmpb[:], swp[:], ss_sb[:, t0 : t0 + TCH])
                    segt = rpool.tile([P, TCH], bf16, tag="segt")
                    nc.vector.tensor_add(segt[:], tmpa[:], tmpb[:])
                    # rms: ms = ones^T @ segt^2 (PE), rstd = exp(-.5 ln(ms/D+eps))
                    # on [1,512] (ACT), broadcast via PE ones-row matmul
                    sq = rpool.tile([P, TCH], bf16, tag="sq")
                    nc.vector.tensor_mul(sq[:], segt[:], segt[:])
                    ms = statp.tile([1, TCH], f32, tag="stat")
                    nc.tensor.matmul(ms[:], ones_c[:], sq[:], start=True, stop=True)
                    lnm = rpool.tile([1, TCH], f32, tag="lnm")
                    nc.scalar.activation(
                        lnm[:], ms[:], AF.Ln, bias=eps_sb[0:1, :], scale=1.0 / D
                    )
                    rstd = rpool.tile([1, TCH], f32r, tag="rstd")
                    nc.scalar.activation(rstd[:], lnm[:], AF.Exp, scale=-0.5)
                    bc_ps = pot.tile([P, TCH], f32, tag="ot")
                    nc.tensor.matmul(bc_ps[:], ones_r[:], rstd[:], start=True, stop=True)
                    nc.vector.tensor_mul(qk_rt[:, fb, t0 : t0 + TCH], segt[:], bc_ps[:])
                if tch == 0:
                    nc.sync.dma_start(wv_sb[:], wv_r)
                    nc.sync.dma_start(mask_sb[:], maskT[:, :, :])
                    nc.sync.dma_start(wo_sb[:], wo_r)
                # V directly in natural [t, d] layout: x-block stationary
                for tb in range(TCH // P):
                    vn = mm.tile([P, TCH], f32, tag="mm")
                    for ko in range(KO):
                        nc.tensor.matmul(
                            vn[:, 0:FK],
                            xt[:, ko, tb * P : (tb + 1) * P],
                            wv_sb[:, ko, :],
                            start=(ko == 0), stop=(ko == KO - 1),
                        )
                    nc.vector.tensor_copy(v_sb[:, tch * 4 + tb, :], vn[:, 0:FK])

                # ---------------- attention span s = tch ----------------
                s = tch
                q0 = s * SPAN
                nkp = 2 * s + 2          # kb pairs
                deferred = []            # per-head normalize chains, emitted at span end
                for h in range(NQ):
                    j = h // 2
                    q_ap = qk_rt[:, h, q0 : q0 + SPAN]
                    ot_ps = pot.tile([P, SPAN], f32, tag="ot")
                    sum_ps = statp.tile([1, SPAN], f32, tag="stat")

                    def scores(p):
                        st2 = spool.tile([P, 2, SPAN], f32, tag="st")
                        for i in range(2):
                            nc.tensor.matmul(
                                st2[:, i, :],
                                qk_rt[:, NQ + j, (2 * p + i) * P : (2 * p + i + 1) * P],
                                q_ap,
                                start=True, stop=True,
                            )
                        return st2

                    st_cur = scores(0)
                    for p in range(nkp):
                        st_nxt = scores(p + 1) if p + 1 < nkp else None
                        pt2 = ppool.tile([P, 2, SPAN], bf16, tag="pt")
                        nc.scalar.activation(pt2[:], st_cur[:], AF.Exp, scale=SCALE)
                        if p >= 2 * s:  # diagonal pairs need the causal mask
                            moff = 2 * (p - 2 * s)
                            nc.vector.tensor_mul(
                                pt2[:], pt2[:], mask_sb[:, moff : moff + 2, :]
                            )
                        for i in range(2):
                            kb = 2 * p + i
                            nc.tensor.matmul(
                                ot_ps[:],
                                v_sb[:, kb, j * D : (j + 1) * D],
                                pt2[:, i, :],
                                start=(kb == 0), stop=(kb == 2 * nkp - 1),
                                skip_group_check=True,
                            )
                            nc.tensor.matmul(
                                sum_ps[:],
                                ones_c[:],
                                pt2[:, i, :],
                                start=(kb == 0), stop=(kb == 2 * nkp - 1),
                                skip_group_check=True,
                            )
                        st_cur = st_nxt
                    # drain ot to SBUF fast on DVE (frees the single ot bank
                    # without using ACT); Ln releases the stat slot right
                    # away; the rest of the normalize chain is deferred to
                    # span end so it never delays the next head's exps
                    otc = apool.tile([P, SPAN], bf16, tag="otc")
                    nc.scalar.copy(otc[:], ot_ps[:])
                    lns = apool.tile([1, SPAN], f32, tag="lns")
                    nc.scalar.activation(lns[:], sum_ps[:], AF.Ln)

                    def normalize(h=h, otc=otc, lns=lns):
                        rec = apool.tile([1, SPAN], f32r, tag="rec")
                        nc.scalar.activation(rec[:], lns[:], AF.Exp, scale=-1.0)
                        bc_ps = mm.tile([P, TCH], f32, tag="mm")
                        nc.tensor.matmul(
                            bc_ps[:], ones_r[:], rec[:], start=True, stop=True
                        )
                        nc.vector.tensor_mul(
                            ot_sb[:, h, q0 : q0 + SPAN], otc[:], bc_ps[:]
                        )

                    deferred.append(normalize)
                for fn in deferred:
                    fn()

                # ---------------- output projection for this span ----------------
                for tb in range(4 * s, 4 * s + 4):
                    for nch in range(C // 512):
                        yps = mm.tile([P, TCH], f32, tag="mm")
                        for h in range(NQ):
                            nc.tensor.matmul(
                                yps[:],
                                ot_sb[:, h, tb * P : (tb + 1) * P],
                                wo_sb[:, h, nch * 512 : (nch + 1) * 512],
                                start=(h == 0), stop=(h == NQ - 1),
                            )
                        ysb = ypool.tile([P, 512], bf16, tag="ysb")
                        nc.vector.tensor_copy(ysb[:], yps[:])
                        nc.sync.dma_start(
                            y[tb * P : (tb + 1) * P, nch * 512 : (nch + 1) * 512],
                            ysb[:],
                        )
    nc.compile()
    n = _force_single_act_table(nc)
    print(f"act-table surgery: kept {n} table loads")
    return nc


_NC_CACHE = None


def _get_nc():
    global _NC_CACHE
    if _NC_CACHE is None:
        _NC_CACHE = build()
    return _NC_CACHE


def _host_inputs(x, cos, sin, wq, wk, wv, wo):
    """Build the 8 per-core input maps."""
    bft = ml_dtypes.bfloat16
    cosT = np.ascontiguousarray(cos[0, :, 0, :].T).astype(np.float32)  # (64, T)
    sinT = np.ascontiguousarray(sin[0, :, 0, :].T).astype(np.float32)
    cc = np.concatenate([cosT, cosT], axis=0).astype(bft)     # (128, T)
    ss = np.concatenate([sinT, -sinT], axis=0).astype(bft)
    # maskT[r][k, q] = 1 if q >= 128*r + k  (within a 512-q span, k-block offset r)
    qidx = np.arange(SPAN)[None, None, :]
    kidx = np.arange(P)[:, None, None]
    ridx = np.arange(4)[None, :, None]
    maskT = (qidx >= P * ridx + kidx).astype(bft)  # (128, 4, 512)

    xTs = [np.ascontiguousarray(x[b].T).astype(bft) for b in range(2)]
    wq16 = wq.astype(bft)
    wk16 = wk.astype(bft)
    wv16 = wv.astype(bft)
    wo16 = wo.astype(bft)
    in_maps = []
    for c in range(8):
        b, tp = divmod(c, 4)
        in_maps.append(
            {
                "xT": xTs[b],
                "wq": np.ascontiguousarray(wq16[:, tp * FQ : (tp + 1) * FQ]),
                "wk": np.ascontiguousarray(wk16[:, tp * FK : (tp + 1) * FK]),
                "wv": np.ascontiguousarray(wv16[:, tp * FK : (tp + 1) * FK]),
                "wo": np.ascontiguousarray(wo16[tp * FQ : (tp + 1) * FQ, :]),
                "cc": cc,
                "ss": ss,
                "maskT": maskT,
            }
        )
    return in_maps


def kernel(x, cos, sin, wq, wk, wv, wo, trace=False):
    x = np.asarray(x, dtype=np.float32)
    cos = np.asarray(cos, dtype=np.float32)
    sin = np.asarray(sin, dtype=np.float32)
    wq = np.asarray(wq, dtype=np.float32)
    wk = np.asarray(wk, dtype=np.float32)
    wv = np.asarray(wv, dtype=np.float32)
    wo = np.asarray(wo, dtype=np.float32)

    nc = _get_nc()
    in_maps = _host_inputs(x, cos, sin, wq, wk, wv, wo)
    res = run_bass_kernel_spmd(nc, in_maps, core_ids=list(range(8)), trace=trace)
    out = np.zeros((2, T, C), dtype=np.float32)
    for c in range(8):
        b = c // 4
        out[b] += np.asarray(res.results[c]["y"], dtype=np.float32)
    if trace:
        return out, res
    return out


# revision 27
# speedup vs baseline: 1.0022x; 1.0008x over previous
"""Causal self-attention (RoPE + QK-RMSNorm, GQA 16q/8kv) Trainium2 Bass kernel.

Sharding: 8 cores = 2 batch x 4 tensor-parallel. Core c handles batch b=c//4 and
q-heads [4*tp, 4*tp+4), kv-heads [2*tp, 2*tp+2) where tp=c%4. Each core returns a
partial (T, C) output = O_heads @ wo[rows of its heads]; host sums the 4 partials
per batch (the "all-reduce after c_proj").

v4: single interleaved pipeline (QKV chunk c -> attention span c -> c_proj of
span c). Partition reductions (RMS ms, softmax denominators) and [1,512]
broadcasts all run on the PE (ones-matmuls, 213ns each) -- GpSimd is avoided
entirely (its ops carry ~1.5us sequencer/semaphore latency). V is computed
directly in natural [t,d] layout (x-block stationary), so no PE transposes.
rstd/recip chains stay on ACT (Ln/Exp); a post-compile pass forces the single
combined ln+exp+copy activation table so there is no table thrash. The single
ot PSUM bank is drained to SBUF by an ACT copy immediately after the AV
accumulation so the next head's matmuls are not blocked by the normalize chain.
"""
import sys
import math

sys.path.insert(0, "/opt/trn_rl_repo")

import numpy as np
import ml_dtypes
import concourse.bacc as bacc
import concourse.mybir as mybir
import concourse.tile as tile
from concourse.bass_utils import run_bass_kernel_spmd

P = 128
T = 2048
C = 2048
KO = C // P          # 16 contraction tiles
D = 128              # head dim
NQ = 4               # q heads per core
NK = 2               # kv heads per core
NF = NQ + NK         # 6 rope/rms feature blocks (4 q + 2 k)
FQ = NQ * D          # 512
FK = NK * D          # 256
TCH = 512            # T-chunk = q-span
NCHUNK = T // TCH    # 4
SPAN = TCH
KB = T // P          # 16 key blocks
SCALE = 1.0 / math.sqrt(D)
EPS = 1.1920929e-07

f32 = mybir.dt.float32
f32r = mybir.dt.float32r
bf16 = mybir.dt.bfloat16

AF = mybir.ActivationFunctionType

# index of 'natural_log_exp_and_others' in act_info.json act_func_sets
ACT_TABLE_LN_EXP = 6


def _force_single_act_table(nc):
    """Replace the compiler's thrashing ACT table loads (alternating
    natural_log / exp_and_others, 1.28us each) with a single load of the
    combined ln+exp+copy table per block."""
    n_kept = 0
    for fn in nc.m.functions:
        for b in fn.blocks:
            newinsts = []
            seen = False
            for inst in b.instructions:
                if isinstance(inst, mybir.InstLoadActFuncSet):
                    if seen:
                        continue
                    inst.act_func_set_id = ACT_TABLE_LN_EXP
                    seen = True
                    n_kept += 1
                newinsts.append(inst)
            b.instructions[:] = newinsts
    return n_kept


def build():
    nc = bacc.Bacc("TRN2", target_bir_lowering=False)
    xT = nc.dram_tensor("xT", (C, T), bf16, kind="ExternalInput")
    wq = nc.dram_tensor("wq", (C, FQ), bf16, kind="ExternalInput")
    wk = nc.dram_tensor("wk", (C, FK), bf16, kind="ExternalInput")
    wv = nc.dram_tensor("wv", (C, FK), bf16, kind="ExternalInput")
    wo = nc.dram_tensor("wo", (FQ, C), bf16, kind="ExternalInput")
    cc = nc.dram_tensor("cc", (P, T), bf16, kind="ExternalInput")    # [cos; cos]
    ss = nc.dram_tensor("ss", (P, T), bf16, kind="ExternalInput")    # [sin; -sin]
    maskT = nc.dram_tensor("maskT", (P, 4, SPAN), bf16, kind="ExternalInput")
    y = nc.dram_tensor("y", (T, C), bf16, kind="ExternalOutput")

    xT_r = xT.rearrange("(ko p) t -> p ko t", p=P)
    wq_r = wq.rearrange("(ko p) f -> p ko f", p=P)
    wk_r = wk.rearrange("(ko p) f -> p ko f", p=P)
    wv_r = wv.rearrange("(ko p) f -> p ko f", p=P)
    wo_r = wo.rearrange("(ko p) n -> p ko n", p=P)

    with tile.TileContext(nc) as tc:
        with (
            tc.tile_pool(name="persist", bufs=1) as persist,
            tc.tile_pool(name="xp", bufs=2) as xpool,
            tc.tile_pool(name="rp", bufs=2) as rpool,
            tc.tile_pool(name="ptp", bufs=3) as ppool,
            tc.tile_pool(name="ap", bufs=4) as apool,
            tc.tile_pool(name="yp", bufs=2) as ypool,
            tc.tile_pool(name="mm", bufs=2, space="PSUM") as mm,
            tc.tile_pool(name="stat", bufs=1, space="PSUM") as statp,
            tc.tile_pool(name="st", bufs=2, space="PSUM") as spool,
            tc.tile_pool(name="ot", bufs=1, space="PSUM") as pot,
        ):
            qk_rt = persist.tile([P, NF, T], bf16, tag="qk_rt")   # roped+normed qT/kT
            v_sb = persist.tile([P, KB, FK], bf16, tag="v_sb")    # V natural [t-part, kb, feat]
            cc_sb = persist.tile([P, T], bf16, tag="cc_sb")
            ss_sb = persist.tile([P, T], bf16, tag="ss_sb")
            ot_sb = persist.tile([P, NQ, T], bf16, tag="ot_sb")
            mask_sb = persist.tile([P, 4, SPAN], bf16, tag="mask_sb")
            wo_sb = persist.tile([P, NQ, C], bf16, tag="wo_sb")
            wq_sb = persist.tile([P, KO, FQ], bf16, tag="wq_sb")
            wk_sb = persist.tile([P, KO, FK], bf16, tag="wk_sb")
            wv_sb = persist.tile([P, KO, FK], bf16, tag="wv_sb")
            eps_sb = persist.tile([P, 1], f32, tag="eps_sb")
            ones_c = persist.tile([P, 1], bf16, tag="ones_c")
            ones_r = persist.tile([1, P], f32r, tag="ones_r")
            ones_rf = persist.tile([1, P], f32, tag="ones_rf")
            nc.vector.memset(eps_sb[:], EPS)
            nc.vector.memset(ones_c[:], 1.0)
            nc.vector.memset(ones_rf[:], 1.0)
            nc.vector.tensor_copy(ones_r[:], ones_rf[:])
            # only what chunk 0's q/k needs goes first; wv/mask/wo are
            # emitted lazily below so the first matmul isn't stuck behind
            # 8MB of weight DMAs
            nc.sync.dma_start(cc_sb[:], cc[:, :])
            nc.sync.dma_start(ss_sb[:], ss[:, :])

            for tch in range(NCHUNK):
                t0 = tch * TCH
                # ---------------- QKV chunk tch ----------------
                xt = xpool.tile([P, KO, TCH], bf16, tag="xt")
                for ko in range(KO):
                    nc.sync.dma_start(xt[:, ko, :], xT_r[:, ko, t0 : t0 + TCH])
                    if tch == 0:
                        # interleave weight slices with x slices so the first
                        # matmuls aren't stuck behind whole-weight DMAs
                        nc.sync.dma_start(wq_sb[:, ko, :], wq_r[:, ko, :])
                        nc.sync.dma_start(wk_sb[:, ko, :], wk_r[:, ko, :])
                # q/k blocks: rope + rms norm into qk_rt
                for fb in range(NF):
                    if fb < NQ:
                        w_ap = wq_sb[:, :, fb * D : (fb + 1) * D]
                    else:
                        w_ap = wk_sb[:, :, (fb - NQ) * D : (fb - NQ + 1) * D]
                    pqk = mm.tile([P, TCH], f32, tag="mm")
                    for ko in range(KO):
                        nc.tensor.matmul(
                            pqk[:], w_ap[:, ko], xt[:, ko, :],
                            start=(ko == 0), stop=(ko == KO - 1),
                        )
                    raw = rpool.tile([P, TCH], bf16, tag="raw")
                    nc.vector.tensor_copy(raw[:], pqk[:])
                    swp = rpool.tile([P, TCH], bf16, tag="swp")
                    nc.sync.dma_start(swp[0:64, :], raw[64:128, :])
                    nc.sync.dma_start(swp[64:128, :], raw[0:64, :])
                    tmpa = rpool.tile([P, TCH], bf16, tag="tmpa")
                    tmpb = rpool.tile([P, TCH], bf16, tag="tmpb")
                    nc.vector.tensor_mul(tmpa[:], raw[:], cc_sb[:, t0 : t0 + TCH])
                    nc.vector.tensor_mul(tmpb[:], swp[:], ss_sb[:, t0 : t0 + TCH])
                    segt = rpool.tile([P, TCH], bf16, tag="segt")
                    nc.vector.tensor_add(segt[:], tmpa[:], tmpb[:])
                    # rms: ms = ones^T @ segt^2 (PE), rstd = exp(-.5 ln(ms/D+eps))
                    # on [1,512] (ACT), broadcast via PE ones-row matmul
                    sq = rpool.tile([P, TCH], bf16, tag="sq")
                    nc.vector.tensor_mul(sq[:], segt[:], segt[:])
                    ms = statp.tile([1, TCH], f32, tag="stat")
                    nc.tensor.matmul(ms[:], ones_c[:], sq[:], start=True, stop=True)
                    lnm = rpool.tile([1, TCH], f32, tag="lnm")
                    nc.scalar.activation(
                        lnm[:], ms[:], AF.Ln, bias=eps_sb[0:1, :], scale=1.0 / D
                    )
                    rstd = rpool.tile([1, TCH], f32r, tag="rstd")
                    nc.scalar.activation(rstd[:], lnm[:], AF.Exp, scale=-0.5)
                    bc_ps = pot.tile([P, TCH], f32, tag="ot")
                    nc.tensor.matmul(bc_ps[:], ones_r[:], rstd[:], start=True, stop=True)
                    nc.vector.tensor_mul(qk_rt[:, fb, t0 : t0 + TCH], segt[:], bc_ps[:])
                if tch == 0:
                    nc.sync.dma_start(wv_sb[:], wv_r)
                    nc.sync.dma_start(mask_sb[:], maskT[:, :, :])
                    nc.sync.dma_start(wo_sb[:], wo_r)
                # V directly in natural [t, d] layout: x-block stationary
                for tb in range(TCH // P):
                    vn = mm.tile([P, TCH], f32, tag="mm")
                    for ko in range(KO):
                        nc.tensor.matmul(
                            vn[:, 0:FK],
                            xt[:, ko, tb * P : (tb + 1) * P],
                            wv_sb[:, ko, :],
                            start=(ko == 0), stop=(ko == KO - 1),
                        )
                    nc.vector.tensor_copy(v_sb[:, tch * 4 + tb, :], vn[:, 0:FK])

                # ---------------- attention span s = tch ----------------
                s = tch
                q0 = s * SPAN
                nkp = 2 * s + 2          # kb pairs
                deferred = []            # per-head normalize chains, emitted at span end
                for h in range(NQ):
                    j = h // 2
                    q_ap = qk_rt[:, h, q0 : q0 + SPAN]
                    ot_ps = pot.tile([P, SPAN], f32, tag="ot")
                    sum_ps = statp.tile([1, SPAN], f32, tag="stat")

                    def scores(p):
                        st2 = spool.tile([P, 2, SPAN], f32, tag="st")
                        for i in range(2):
                            nc.tensor.matmul(
                                st2[:, i, :],
                                qk_rt[:, NQ + j, (2 * p + i) * P : (2 * p + i + 1) * P],
                                q_ap,
                                start=True, stop=True,
                            )
                        return st2

                    st_cur = scores(0)
                    for p in range(nkp):
                        st_nxt = scores(p + 1) if p + 1 < nkp else None
                        pt2 = ppool.tile([P, 2, SPAN], bf16, tag="pt")
                        nc.scalar.activation(pt2[:], st_cur[:], AF.Exp, scale=SCALE)
                        if p >= 2 * s:  # diagonal pairs need the causal mask
                            moff = 2 * (p - 2 * s)
                            nc.vector.tensor_mul(
                                pt2[:], pt2[:], mask_sb[:, moff : moff + 2, :]
                            )
                        for i in range(2):
                            kb = 2 * p + i
                            nc.tensor.matmul(
                                ot_ps[:],
                                v_sb[:, kb, j * D : (j + 1) * D],
                                pt2[:, i, :],
                                start=(kb == 0), stop=(kb == 2 * nkp - 1),
                                skip_group_check=True,
                            )
                            nc.tensor.matmul(
                                sum_ps[:],
                                ones_c[:],
                                pt2[:, i, :],
                                start=(kb == 0), stop=(kb == 2 * nkp - 1),
                                skip_group_check=True,
                            )
                        st_cur = st_nxt
                    # drain ot to SBUF fast (frees the single ot bank), then
                    # normalize off the PE-critical path
                    otc = apool.tile([P, SPAN], bf16, tag="otc")
                    nc.scalar.copy(otc[:], ot_ps[:])
                    lns = apool.tile([1, SPAN], f32, tag="lns")
                    nc.scalar.activation(lns[:], sum_ps[:], AF.Ln)
                    rec = apool.tile([1, SPAN], f32r, tag="rec")
                    nc.scalar.activation(rec[:], lns[:], AF.Exp, scale=-1.0)
                    bc_ps = mm.tile([P, TCH], f32, tag="mm")
                    nc.tensor.matmul(bc_ps[:], ones_r[:], rec[:], start=True, stop=True)
                    nc.vector.tensor_mul(
                        ot_sb[:, h, q0 : q0 + SPAN], otc[:], bc_ps[:]
                    )

                # ---------------- output projection for this span ----------------
                for tb in range(4 * s, 4 * s + 4):
                    for nch in range(C // 512):
                        yps = mm.tile([P, TCH], f32, tag="mm")
                        for h in range(NQ):
                            nc.tensor.matmul(
                                yps[:],
                                ot_sb[:, h, tb * P : (tb + 1) * P],
                                wo_sb[:, h, nch * 512 : (nch + 1) * 512],
                                start=(h == 0), stop=(h == NQ - 1),
                            )
                        ysb = ypool.tile([P, 512], bf16, tag="ysb")
                        nc.vector.tensor_copy(ysb[:], yps[:])
                        nc.sync.dma_start(
                            y[tb * P : (tb + 1) * P, nch * 512 : (nch + 1) * 512],
                            ysb[:],
                        )
    nc.compile()
    n = _force_single_act_table(nc)
    print(f"act-table surgery: kept {n} table loads")
    return nc


_NC_CACHE = None


def _get_nc():
    global _NC_CACHE
    if _NC_CACHE is None:
        _NC_CACHE = build()
    return _NC_CACHE


def _host_inputs(x, cos, sin, wq, wk, wv, wo):
    """Build the 8 per-core input maps."""
    bft = ml_dtypes.bfloat16
    cosT = np.ascontiguousarray(cos[0, :, 0, :].T).astype(np.float32)  # (64, T)
    sinT = np.ascontiguousarray(sin[0, :, 0, :].T).astype(np.float32)
    cc = np.concatenate([cosT, cosT], axis=0).astype(bft)     # (128, T)
    ss = np.concatenate([sinT, -sinT], axis=0).astype(bft)
    # maskT[r][k, q] = 1 if q >= 128*r + k  (within a 512-q span, k-block offset r)
    qidx = np.arange(SPAN)[None, None, :]
    kidx = np.arange(P)[:, None, None]
    ridx = np.arange(4)[None, :, None]
    maskT = (qidx >= P * ridx + kidx).astype(bft)  # (128, 4, 512)

    xTs = [np.ascontiguousarray(x[b].T).astype(bft) for b in range(2)]
    wq16 = wq.astype(bft)
    wk16 = wk.astype(bft)
    wv16 = wv.astype(bft)
    wo16 = wo.astype(bft)
    in_maps = []
    for c in range(8):
        b, tp = divmod(c, 4)
        in_maps.append(
            {
                "xT": xTs[b],
                "wq": np.ascontiguousarray(wq16[:, tp * FQ : (tp + 1) * FQ]),
                "wk": np.ascontiguousarray(wk16[:, tp * FK : (tp + 1) * FK]),
                "wv": np.ascontiguousarray(wv16[:, tp * FK : (tp + 1) * FK]),
                "wo": np.ascontiguousarray(wo16[tp * FQ : (tp + 1) * FQ, :]),
                "cc": cc,
                "ss": ss,
                "maskT": maskT,
            }
        )
    return in_maps


def kernel(x, cos, sin, wq, wk, wv, wo, trace=False):
    x = np.asarray(x, dtype=np.float32)
    cos = np.asarray(cos, dtype=np.float32)
    sin = np.asarray(sin, dtype=np.float32)
    wq = np.asarray(wq, dtype=np.float32)
    wk = np.asarray(wk, dtype=np.float32)
    wv = np.asarray(wv, dtype=np.float32)
    wo = np.asarray(wo, dtype=np.float32)

    nc = _get_nc()
    in_maps = _host_inputs(x, cos, sin, wq, wk, wv, wo)
    res = run_bass_kernel_spmd(nc, in_maps, core_ids=list(range(8)), trace=trace)
    out = np.zeros((2, T, C), dtype=np.float32)
    for c in range(8):
        b = c // 4
        out[b] += np.asarray(res.results[c]["y"], dtype=np.float32)
    if trace:
        return out, res
    return out


# revision 31
# speedup vs baseline: 1.0205x; 1.0183x over previous
"""Causal self-attention (RoPE + QK-RMSNorm, GQA 16q/8kv) Trainium2 Bass kernel.

Sharding: 8 cores = 2 batch x 4 tensor-parallel. Core c handles batch b=c//4 and
q-heads [4*tp, 4*tp+4), kv-heads [2*tp, 2*tp+2) where tp=c%4. Each core returns a
partial (T, C) output = O_heads @ wo[rows of its heads]; host sums the 4 partials
per batch (the "all-reduce after c_proj").

v4: single interleaved pipeline (QKV chunk c -> attention span c -> c_proj of
span c). Partition reductions (RMS ms, softmax denominators) and [1,512]
broadcasts all run on the PE (ones-matmuls, 213ns each) -- GpSimd is avoided
entirely (its ops carry ~1.5us sequencer/semaphore latency). V is computed
directly in natural [t,d] layout (x-block stationary), so no PE transposes.
rstd/recip chains stay on ACT (Ln/Exp); a post-compile pass forces the single
combined ln+exp+copy activation table so there is no table thrash. The single
ot PSUM bank is drained to SBUF by an ACT copy immediately after the AV
accumulation so the next head's matmuls are not blocked by the normalize chain.
"""
import sys
import math

sys.path.insert(0, "/opt/trn_rl_repo")

import numpy as np
import ml_dtypes
import concourse.bacc as bacc
import concourse.mybir as mybir
import concourse.tile as tile
from concourse.bass_utils import run_bass_kernel_spmd

P = 128
T = 2048
C = 2048
KO = C // P          # 16 contraction tiles
D = 128              # head dim
NQ = 4               # q heads per core
NK = 2               # kv heads per core
NF = NQ + NK         # 6 rope/rms feature blocks (4 q + 2 k)
FQ = NQ * D          # 512
FK = NK * D          # 256
TCH = 512            # T-chunk = q-span
NCHUNK = T // TCH    # 4
SPAN = TCH
KB = T // P          # 16 key blocks
SCALE = 1.0 / math.sqrt(D)
EPS = 1.1920929e-07

f32 = mybir.dt.float32
f32r = mybir.dt.float32r
bf16 = mybir.dt.bfloat16

AF = mybir.ActivationFunctionType

# index of 'natural_log_exp_and_others' in act_info.json act_func_sets
ACT_TABLE_LN_EXP = 6


def _force_single_act_table(nc):
    """Replace the compiler's thrashing ACT table loads (alternating
    natural_log / exp_and_others, 1.28us each) with a single load of the
    combined ln+exp+copy table per block."""
    n_kept = 0
    for fn in nc.m.functions:
        for b in fn.blocks:
            newinsts = []
            seen = False
            for inst in b.instructions:
                if isinstance(inst, mybir.InstLoadActFuncSet):
                    if seen:
                        continue
                    inst.act_func_set_id = ACT_TABLE_LN_EXP
                    seen = True
                    n_kept += 1
                newinsts.append(inst)
            b.instructions[:] = newinsts
    return n_kept


def build():
    nc = bacc.Bacc("TRN2", target_bir_lowering=False)
    xT = nc.dram_tensor("xT", (C, T), bf16, kind="ExternalInput")
    wq = nc.dram_tensor("wq", (C, FQ), bf16, kind="ExternalInput")
    wk = nc.dram_tensor("wk", (C, FK), bf16, kind="ExternalInput")
    wv = nc.dram_tensor("wv", (C, FK), bf16, kind="ExternalInput")
    wo = nc.dram_tensor("wo", (FQ, C), bf16, kind="ExternalInput")
    cc = nc.dram_tensor("cc", (P, T), bf16, kind="ExternalInput")    # [cos; cos]
    ss = nc.dram_tensor("ss", (P, T), bf16, kind="ExternalInput")    # [sin; -sin]
    maskT = nc.dram_tensor("maskT", (P, 4, SPAN), bf16, kind="ExternalInput")
    y = nc.dram_tensor("y", (T, C), bf16, kind="ExternalOutput")

    xT_r = xT.rearrange("(ko p) t -> p ko t", p=P)
    wq_r = wq.rearrange("(ko p) f -> p ko f", p=P)
    wk_r = wk.rearrange("(ko p) f -> p ko f", p=P)
    wv_r = wv.rearrange("(ko p) f -> p ko f", p=P)
    wo_r = wo.rearrange("(ko p) n -> p ko n", p=P)

    with tile.TileContext(nc) as tc:
        with (
            tc.tile_pool(name="persist", bufs=1) as persist,
            tc.tile_pool(name="xp", bufs=2) as xpool,
            tc.tile_pool(name="rp", bufs=2) as rpool,
            tc.tile_pool(name="ptp", bufs=3) as ppool,
            tc.tile_pool(name="ap", bufs=2) as apool,
            tc.tile_pool(name="yp", bufs=2) as ypool,
            tc.tile_pool(name="mm", bufs=2, space="PSUM") as mm,
            tc.tile_pool(name="stat", bufs=1, space="PSUM") as statp,
            tc.tile_pool(name="st", bufs=2, space="PSUM") as spool,
            tc.tile_pool(name="ot", bufs=1, space="PSUM") as pot,
        ):
            qk_rt = persist.tile([P, NF, T], bf16, tag="qk_rt")   # roped+normed qT/kT
            v_sb = persist.tile([P, KB, FK], bf16, tag="v_sb")    # V natural [t-part, kb, feat]
            cc_sb = persist.tile([P, T], bf16, tag="cc_sb")
            ss_sb = persist.tile([P, T], bf16, tag="ss_sb")
            ot_sb = persist.tile([P, NQ, T], bf16, tag="ot_sb")
            mask_sb = persist.tile([P, 4, SPAN], bf16, tag="mask_sb")
            wo_sb = persist.tile([P, NQ, C], bf16, tag="wo_sb")
            wq_sb = persist.tile([P, KO, FQ], bf16, tag="wq_sb")
            wk_sb = persist.tile([P, KO, FK], bf16, tag="wk_sb")
            wv_sb = persist.tile([P, KO, FK], bf16, tag="wv_sb")
            eps_sb = persist.tile([P, 1], f32, tag="eps_sb")
            ones_c = persist.tile([P, 1], bf16, tag="ones_c")
            ones_r = persist.tile([1, P], f32r, tag="ones_r")
            ones_rf = persist.tile([1, P], f32, tag="ones_rf")
            nc.vector.memset(eps_sb[:], EPS)
            nc.vector.memset(ones_c[:], 1.0)
            nc.vector.memset(ones_rf[:], 1.0)
            nc.vector.tensor_copy(ones_r[:], ones_rf[:])
            # only what chunk 0's q/k needs goes first; wv/mask/wo are
            # emitted lazily below so the first matmul isn't stuck behind
            # 8MB of weight DMAs
            nc.sync.dma_start(wq_sb[:], wq_r)
            nc.sync.dma_start(wk_sb[:], wk_r)
            nc.sync.dma_start(cc_sb[:], cc[:, :])
            nc.sync.dma_start(ss_sb[:], ss[:, :])

            for tch in range(NCHUNK):
                t0 = tch * TCH
                # ---------------- QKV chunk tch ----------------
                xt = xpool.tile([P, KO, TCH], bf16, tag="xt")
                for ko in range(KO):
                    nc.sync.dma_start(xt[:, ko, :], xT_r[:, ko, t0 : t0 + TCH])
                # q/k blocks: rope + rms norm into qk_rt
                for fb in range(NF):
                    if fb < NQ:
                        w_ap = wq_sb[:, :, fb * D : (fb + 1) * D]
                    else:
                        w_ap = wk_sb[:, :, (fb - NQ) * D : (fb - NQ + 1) * D]
                    pqk = mm.tile([P, TCH], f32, tag="mm")
                    for ko in range(KO):
                        nc.tensor.matmul(
                            pqk[:], w_ap[:, ko], xt[:, ko, :],
                            start=(ko == 0), stop=(ko == KO - 1),
                        )
                    raw = rpool.tile([P, TCH], bf16, tag="raw")
                    nc.vector.tensor_copy(raw[:], pqk[:])
                    swp = rpool.tile([P, TCH], bf16, tag="swp")
                    nc.sync.dma_start(swp[0:64, :], raw[64:128, :])
                    nc.sync.dma_start(swp[64:128, :], raw[0:64, :])
                    tmpa = rpool.tile([P, TCH], bf16, tag="tmpa")
                    tmpb = rpool.tile([P, TCH], bf16, tag="tmpb")
                    nc.vector.tensor_mul(tmpa[:], raw[:], cc_sb[:, t0 : t0 + TCH])
                    nc.vector.tensor_mul(tmpb[:], swp[:], ss_sb[:, t0 : t0 + TCH])
                    segt = rpool.tile([P, TCH], bf16, tag="segt")
                    nc.vector.tensor_add(segt[:], tmpa[:], tmpb[:])
                    # rms: ms = ones^T @ segt^2 (PE), rstd = exp(-.5 ln(ms/D+eps))
                    # on [1,512] (ACT), broadcast via PE ones-row matmul
                    sq = rpool.tile([P, TCH], bf16, tag="sq")
                    nc.vector.tensor_mul(sq[:], segt[:], segt[:])
                    ms = statp.tile([1, TCH], f32, tag="stat")
                    nc.tensor.matmul(ms[:], ones_c[:], sq[:], start=True, stop=True)
                    lnm = rpool.tile([1, TCH], f32, tag="lnm")
                    nc.scalar.activation(
                        lnm[:], ms[:], AF.Ln, bias=eps_sb[0:1, :], scale=1.0 / D
                    )
                    rstd = rpool.tile([1, TCH], f32r, tag="rstd")
                    nc.scalar.activation(rstd[:], lnm[:], AF.Exp, scale=-0.5)
                    bc_ps = pot.tile([P, TCH], f32, tag="ot")
                    nc.tensor.matmul(bc_ps[:], ones_r[:], rstd[:], start=True, stop=True)
                    nc.vector.tensor_mul(qk_rt[:, fb, t0 : t0 + TCH], segt[:], bc_ps[:])
                if tch == 0:
                    nc.sync.dma_start(wv_sb[:], wv_r)
                    nc.sync.dma_start(mask_sb[:], maskT[:, :, :])
                    nc.sync.dma_start(wo_sb[:], wo_r)
                # V directly in natural [t, d] layout: x-block stationary
                for tb in range(TCH // P):
                    vn = mm.tile([P, TCH], f32, tag="mm")
                    for ko in range(KO):
                        nc.tensor.matmul(
                            vn[:, 0:FK],
                            xt[:, ko, tb * P : (tb + 1) * P],
                            wv_sb[:, ko, :],
                            start=(ko == 0), stop=(ko == KO - 1),
                        )
                    nc.vector.tensor_copy(v_sb[:, tch * 4 + tb, :], vn[:, 0:FK])

                # ---------------- attention span s = tch ----------------
                s = tch
                q0 = s * SPAN
                nkp = 2 * s + 2          # kb pairs
                for h in range(NQ):
                    j = h // 2
                    q_ap = qk_rt[:, h, q0 : q0 + SPAN]
                    ot_ps = pot.tile([P, SPAN], f32, tag="ot")
                    sum_ps = statp.tile([1, SPAN], f32, tag="stat")

                    def scores(p):
                        st2 = spool.tile([P, 2, SPAN], f32, tag="st")
                        for i in range(2):
                            nc.tensor.matmul(
                                st2[:, i, :],
                                qk_rt[:, NQ + j, (2 * p + i) * P : (2 * p + i + 1) * P],
                                q_ap,
                                start=True, stop=True,
                            )
                        return st2

                    st_cur = scores(0)
                    for p in range(nkp):
                        st_nxt = scores(p + 1) if p + 1 < nkp else None
                        pt2 = ppool.tile([P, 2, SPAN], bf16, tag="pt")
                        nc.scalar.activation(pt2[:], st_cur[:], AF.Exp, scale=SCALE)
                        if p >= 2 * s:  # diagonal pairs need the causal mask
                            moff = 2 * (p - 2 * s)
                            nc.vector.tensor_mul(
                                pt2[:], pt2[:], mask_sb[:, moff : moff + 2, :]
                            )
                        for i in range(2):
                            kb = 2 * p + i
                            nc.tensor.matmul(
                                ot_ps[:],
                                v_sb[:, kb, j * D : (j + 1) * D],
                                pt2[:, i, :],
                                start=(kb == 0), stop=(kb == 2 * nkp - 1),
                                skip_group_check=True,
                            )
                            nc.tensor.matmul(
                                sum_ps[:],
                                ones_c[:],
                                pt2[:, i, :],
                                start=(kb == 0), stop=(kb == 2 * nkp - 1),
                                skip_group_check=True,
                            )
                        st_cur = st_nxt
                    # drain ot to SBUF fast (frees the single ot bank), then
                    # normalize off the PE-critical path
                    otc = apool.tile([P, SPAN], bf16, tag="otc")
                    nc.scalar.copy(otc[:], ot_ps[:])
                    lns = apool.tile([1, SPAN], f32, tag="lns")
                    nc.scalar.activation(lns[:], sum_ps[:], AF.Ln)
                    rec = apool.tile([1, SPAN], f32r, tag="rec")
                    nc.scalar.activation(rec[:], lns[:], AF.Exp, scale=-1.0)
                    bc_ps = mm.tile([P, TCH], f32, tag="mm")
                    nc.tensor.matmul(bc_ps[:], ones_r[:], rec[:], start=True, stop=True)
                    nc.vector.tensor_mul(
                        ot_sb[:, h, q0 : q0 + SPAN], otc[:], bc_ps[:]
                    )

                # ---------------- output projection for this span ----------------
                for tb in range(4 * s, 4 * s + 4):
                    for nch in range(C // 512):
                        yps = mm.tile([P, TCH], f32, tag="mm")
                        for h in range(NQ):
                            nc.tensor.matmul(
                                yps[:],
                                ot_sb[:, h, tb * P : (tb + 1) * P],
                                wo_sb[:, h, nch * 512 : (nch + 1) * 512],
                                start=(h == 0), stop=(h == NQ - 1),
                            )
                        ysb = ypool.tile([P, 512], bf16, tag="ysb")
                        nc.vector.tensor_copy(ysb[:], yps[:])
                        nc.sync.dma_start(
                            y[tb * P : (tb + 1) * P, nch * 512 : (nch + 1) * 512],
                            ysb[:],
                        )
    nc.compile()
    n = _force_single_act_table(nc)
    print(f"act-table surgery: kept {n} table loads")
    return nc


_NC_CACHE = None


def _get_nc():
    global _NC_CACHE
    if _NC_CACHE is None:
        _NC_CACHE = build()
    return _NC_CACHE


def _host_inputs(x, cos, sin, wq, wk, wv, wo):
    """Build the 8 per-core input maps."""
    bft = ml_dtypes.bfloat16
    cosT = np.ascontiguousarray(cos[0, :, 0, :].T).astype(np.float32)  # (64, T)
    sinT = np.ascontiguousarray(sin[0, :, 0, :].T).astype(np.float32)
    cc = np.concatenate([cosT, cosT], axis=0).astype(bft)     # (128, T)
    ss = np.concatenate([sinT, -sinT], axis=0).astype(bft)
    # maskT[r][k, q] = 1 if q >= 128*r + k  (within a 512-q span, k-block offset r)
    qidx = np.arange(SPAN)[None, None, :]
    kidx = np.arange(P)[:, None, None]
    ridx = np.arange(4)[None, :, None]
    maskT = (qidx >= P * ridx + kidx).astype(bft)  # (128, 4, 512)

    xTs = [np.ascontiguousarray(x[b].T).astype(bft) for b in range(2)]
    wq16 = wq.astype(bft)
    wk16 = wk.astype(bft)
    wv16 = wv.astype(bft)
    wo16 = wo.astype(bft)
    in_maps = []
    for c in range(8):
        b, tp = divmod(c, 4)
        in_maps.append(
            {
                "xT": xTs[b],
                "wq": np.ascontiguousarray(wq16[:, tp * FQ : (tp + 1) * FQ]),
                "wk": np.ascontiguousarray(wk16[:, tp * FK : (tp + 1) * FK]),
                "wv": np.ascontiguousarray(wv16[:, tp * FK : (tp + 1) * FK]),
                "wo": np.ascontiguousarray(wo16[tp * FQ : (tp + 1) * FQ, :]),
                "cc": cc,
                "ss": ss,
                "maskT": maskT,
            }
        )
    return in_maps


def kernel(x, cos, sin, wq, wk, wv, wo, trace=False):
    x = np.asarray(x, dtype=np.float32)
    cos = np.asarray(cos, dtype=np.float32)
    sin = np.asarray(sin, dtype=np.float32)
    wq = np.asarray(wq, dtype=np.float32)
    wk = np.asarray(wk, dtype=np.float32)
    wv = np.asarray(wv, dtype=np.float32)
    wo = np.asarray(wo, dtype=np.float32)

    nc = _get_nc()
    in_maps = _host_inputs(x, cos, sin, wq, wk, wv, wo)
    res = run_bass_kernel_spmd(nc, in_maps, core_ids=list(range(8)), trace=trace)
    out = np.zeros((2, T, C), dtype=np.float32)
    for c in range(8):
        b = c // 4
        out[b] += np.asarray(res.results[c]["y"], dtype=np.float32)
    if trace:
        return out, res
    return out


# revision 34
# speedup vs baseline: 1.0849x; 1.0631x over previous
"""Causal self-attention (RoPE + QK-RMSNorm, GQA 16q/8kv) Trainium2 Bass kernel.

Sharding: 8 cores = 2 batch x 4 tensor-parallel. Core c handles batch b=c//4 and
q-heads [4*tp, 4*tp+4), kv-heads [2*tp, 2*tp+2) where tp=c%4. Each core returns a
partial (T, C) output = O_heads @ wo[rows of its heads]; host sums the 4 partials
per batch (the "all-reduce after c_proj").

v4: single interleaved pipeline (QKV chunk c -> attention span c -> c_proj of
span c). Partition reductions (RMS ms, softmax denominators) and [1,512]
broadcasts all run on the PE (ones-matmuls, 213ns each) -- GpSimd is avoided
entirely (its ops carry ~1.5us sequencer/semaphore latency). V is computed
directly in natural [t,d] layout (x-block stationary), so no PE transposes.
rstd/recip chains stay on ACT (Ln/Exp); a post-compile pass forces the single
combined ln+exp+copy activation table so there is no table thrash. The single
ot PSUM bank is drained to SBUF by an ACT copy immediately after the AV
accumulation so the next head's matmuls are not blocked by the normalize chain.
"""
import sys
import math

sys.path.insert(0, "/opt/trn_rl_repo")

import numpy as np
import ml_dtypes
import concourse.bacc as bacc
import concourse.mybir as mybir
import concourse.tile as tile
from concourse.bass_utils import run_bass_kernel_spmd

P = 128
T = 2048
C = 2048
KO = C // P          # 16 contraction tiles
D = 128              # head dim
NQ = 4               # q heads per core
NK = 2               # kv heads per core
NF = NQ + NK         # 6 rope/rms feature blocks (4 q + 2 k)
FQ = NQ * D          # 512
FK = NK * D          # 256
TCH = 512            # T-chunk = q-span
NCHUNK = T // TCH    # 4
SPAN = TCH
KB = T // P          # 16 key blocks
SCALE = 1.0 / math.sqrt(D)
EPS = 1.1920929e-07

f32 = mybir.dt.float32
f32r = mybir.dt.float32r
bf16 = mybir.dt.bfloat16

AF = mybir.ActivationFunctionType

# index of 'natural_log_exp_and_others' in act_info.json act_func_sets
ACT_TABLE_LN_EXP = 6


def _force_single_act_table(nc):
    """Replace the compiler's thrashing ACT table loads (alternating
    natural_log / exp_and_others, 1.28us each) with a single load of the
    combined ln+exp+copy table per block."""
    n_kept = 0
    for fn in nc.m.functions:
        for b in fn.blocks:
            newinsts = []
            seen = False
            for inst in b.instructions:
                if isinstance(inst, mybir.InstLoadActFuncSet):
                    if seen:
                        continue
                    inst.act_func_set_id = ACT_TABLE_LN_EXP
                    seen = True
                    n_kept += 1
                newinsts.append(inst)
            b.instructions[:] = newinsts
    return n_kept


def build():
    nc = bacc.Bacc("TRN2", target_bir_lowering=False)
    xT = nc.dram_tensor("xT", (C, T), bf16, kind="ExternalInput")
    wq = nc.dram_tensor("wq", (C, FQ), bf16, kind="ExternalInput")
    wk = nc.dram_tensor("wk", (C, FK), bf16, kind="ExternalInput")
    wv = nc.dram_tensor("wv", (C, FK), bf16, kind="ExternalInput")
    wo = nc.dram_tensor("wo", (FQ, C), bf16, kind="ExternalInput")
    cc = nc.dram_tensor("cc", (P, T), bf16, kind="ExternalInput")    # [cos; cos]
    ss = nc.dram_tensor("ss", (P, T), bf16, kind="ExternalInput")    # [sin; -sin]
    maskT = nc.dram_tensor("maskT", (P, 4, SPAN), bf16, kind="ExternalInput")
    y = nc.dram_tensor("y", (T, C), bf16, kind="ExternalOutput")

    xT_r = xT.rearrange("(ko p) t -> p ko t", p=P)
    wq_r = wq.rearrange("(ko p) f -> p ko f", p=P)
    wk_r = wk.rearrange("(ko p) f -> p ko f", p=P)
    wv_r = wv.rearrange("(ko p) f -> p ko f", p=P)
    wo_r = wo.rearrange("(ko p) n -> p ko n", p=P)

    with tile.TileContext(nc) as tc:
        with (
            tc.tile_pool(name="persist", bufs=1) as persist,
            tc.tile_pool(name="xp", bufs=2) as xpool,
            tc.tile_pool(name="rp", bufs=2) as rpool,
            tc.tile_pool(name="ptp", bufs=3) as ppool,
            tc.tile_pool(name="ap", bufs=2) as apool,
            tc.tile_pool(name="yp", bufs=2) as ypool,
            tc.tile_pool(name="mm", bufs=2, space="PSUM") as mm,
            tc.tile_pool(name="stat", bufs=1, space="PSUM") as statp,
            tc.tile_pool(name="st", bufs=2, space="PSUM") as spool,
            tc.tile_pool(name="ot", bufs=1, space="PSUM") as pot,
        ):
            qk_rt = persist.tile([P, NF, T], bf16, tag="qk_rt")   # roped+normed qT/kT
            v_sb = persist.tile([P, KB, FK], bf16, tag="v_sb")    # V natural [t-part, kb, feat]
            cc_sb = persist.tile([P, T], bf16, tag="cc_sb")
            ss_sb = persist.tile([P, T], bf16, tag="ss_sb")
            ot_sb = persist.tile([P, NQ, T], bf16, tag="ot_sb")
            mask_sb = persist.tile([P, 4, SPAN], bf16, tag="mask_sb")
            wo_sb = persist.tile([P, NQ, C], bf16, tag="wo_sb")
            wq_sb = persist.tile([P, KO, FQ], bf16, tag="wq_sb")
            wk_sb = persist.tile([P, KO, FK], bf16, tag="wk_sb")
            wv_sb = persist.tile([P, KO, FK], bf16, tag="wv_sb")
            eps_sb = persist.tile([P, 1], f32, tag="eps_sb")
            ones_c = persist.tile([P, 1], bf16, tag="ones_c")
            ones_r = persist.tile([1, P], f32r, tag="ones_r")
            ones_rf = persist.tile([1, P], f32, tag="ones_rf")
            nc.vector.memset(eps_sb[:], EPS)
            nc.vector.memset(ones_c[:], 1.0)
            nc.vector.memset(ones_rf[:], 1.0)
            nc.vector.tensor_copy(ones_r[:], ones_rf[:])
            # only what chunk 0's q/k needs goes first; wv/mask/wo are
            # emitted lazily below so the first matmul isn't stuck behind
            # 8MB of weight DMAs
            nc.sync.dma_start(wq_sb[:], wq_r)
            nc.sync.dma_start(wk_sb[:], wk_r)
            nc.sync.dma_start(cc_sb[:], cc[:, :])
            nc.sync.dma_start(ss_sb[:], ss[:, :])

            def proj_group(tb, nch):
                def emit():
                    yps = mm.tile([P, TCH], f32, tag="mm")
                    for h in range(NQ):
                        nc.tensor.matmul(
                            yps[:],
                            ot_sb[:, h, tb * P : (tb + 1) * P],
                            wo_sb[:, h, nch * 512 : (nch + 1) * 512],
                            start=(h == 0), stop=(h == NQ - 1),
                        )
                    ysb = ypool.tile([P, 512], bf16, tag="ysb")
                    nc.vector.tensor_copy(ysb[:], yps[:])
                    nc.sync.dma_start(
                        y[tb * P : (tb + 1) * P, nch * 512 : (nch + 1) * 512],
                        ysb[:],
                    )
                return emit

            pending_proj = []

            for tch in range(NCHUNK):
                t0 = tch * TCH
                # ---------------- QKV chunk tch ----------------
                xt = xpool.tile([P, KO, TCH], bf16, tag="xt")
                for ko in range(KO):
                    nc.sync.dma_start(xt[:, ko, :], xT_r[:, ko, t0 : t0 + TCH])
                # q/k blocks: rope + rms norm into qk_rt
                for fb in range(NF):
                    if fb < NQ:
                        w_ap = wq_sb[:, :, fb * D : (fb + 1) * D]
                    else:
                        w_ap = wk_sb[:, :, (fb - NQ) * D : (fb - NQ + 1) * D]
                    pqk = mm.tile([P, TCH], f32, tag="mm")
                    for ko in range(KO):
                        nc.tensor.matmul(
                            pqk[:], w_ap[:, ko], xt[:, ko, :],
                            start=(ko == 0), stop=(ko == KO - 1),
                        )
                    raw = rpool.tile([P, TCH], bf16, tag="raw")
                    nc.vector.tensor_copy(raw[:], pqk[:])
                    swp = rpool.tile([P, TCH], bf16, tag="swp")
                    nc.sync.dma_start(swp[0:64, :], raw[64:128, :])
                    nc.sync.dma_start(swp[64:128, :], raw[0:64, :])
                    tmpa = rpool.tile([P, TCH], bf16, tag="tmpa")
                    tmpb = rpool.tile([P, TCH], bf16, tag="tmpb")
                    nc.vector.tensor_mul(tmpa[:], raw[:], cc_sb[:, t0 : t0 + TCH])
                    nc.vector.tensor_mul(tmpb[:], swp[:], ss_sb[:, t0 : t0 + TCH])
                    segt = rpool.tile([P, TCH], bf16, tag="segt")
                    nc.vector.tensor_add(segt[:], tmpa[:], tmpb[:])
                    # rms: ms = ones^T @ segt^2 (PE), rstd = exp(-.5 ln(ms/D+eps))
                    # on [1,512] (ACT), broadcast via PE ones-row matmul
                    sq = rpool.tile([P, TCH], bf16, tag="sq")
                    nc.vector.tensor_mul(sq[:], segt[:], segt[:])
                    ms = statp.tile([1, TCH], f32, tag="stat")
                    nc.tensor.matmul(ms[:], ones_c[:], sq[:], start=True, stop=True)
                    lnm = rpool.tile([1, TCH], f32, tag="lnm")
                    nc.scalar.activation(
                        lnm[:], ms[:], AF.Ln, bias=eps_sb[0:1, :], scale=1.0 / D
                    )
                    rstd = rpool.tile([1, TCH], f32r, tag="rstd")
                    nc.scalar.activation(rstd[:], lnm[:], AF.Exp, scale=-0.5)
                    bc_ps = pot.tile([P, TCH], f32, tag="ot")
                    nc.tensor.matmul(bc_ps[:], ones_r[:], rstd[:], start=True, stop=True)
                    nc.vector.tensor_mul(qk_rt[:, fb, t0 : t0 + TCH], segt[:], bc_ps[:])
                if tch == 0:
                    nc.sync.dma_start(wv_sb[:], wv_r)
                    nc.sync.dma_start(mask_sb[:], maskT[:, :, :])
                    nc.sync.dma_start(wo_sb[:], wo_r)
                # V directly in natural [t, d] layout: x-block stationary
                for tb in range(TCH // P):
                    vn = mm.tile([P, TCH], f32, tag="mm")
                    for ko in range(KO):
                        nc.tensor.matmul(
                            vn[:, 0:FK],
                            xt[:, ko, tb * P : (tb + 1) * P],
                            wv_sb[:, ko, :],
                            start=(ko == 0), stop=(ko == KO - 1),
                        )
                    nc.vector.tensor_copy(v_sb[:, tch * 4 + tb, :], vn[:, 0:FK])

                # ---------------- attention span s = tch ----------------
                s = tch
                q0 = s * SPAN
                nkp = 2 * s + 2          # kb pairs
                for h in range(NQ):
                    j = h // 2
                    q_ap = qk_rt[:, h, q0 : q0 + SPAN]
                    ot_ps = pot.tile([P, SPAN], f32, tag="ot")
                    sum_ps = statp.tile([1, SPAN], f32, tag="stat")

                    def scores(p):
                        st2 = spool.tile([P, 2, SPAN], f32, tag="st")
                        for i in range(2):
                            nc.tensor.matmul(
                                st2[:, i, :],
                                qk_rt[:, NQ + j, (2 * p + i) * P : (2 * p + i + 1) * P],
                                q_ap,
                                start=True, stop=True,
                            )
                        return st2

                    st_cur = scores(0)
                    for p in range(nkp):
                        st_nxt = scores(p + 1) if p + 1 < nkp else None
                        pt2 = ppool.tile([P, 2, SPAN], bf16, tag="pt")
                        nc.scalar.activation(pt2[:], st_cur[:], AF.Exp, scale=SCALE)
                        if p >= 2 * s:  # diagonal pairs need the causal mask
                            moff = 2 * (p - 2 * s)
                            nc.vector.tensor_mul(
                                pt2[:], pt2[:], mask_sb[:, moff : moff + 2, :]
                            )
                        for i in range(2):
                            kb = 2 * p + i
                            nc.tensor.matmul(
                                ot_ps[:],
                                v_sb[:, kb, j * D : (j + 1) * D],
                                pt2[:, i, :],
                                start=(kb == 0), stop=(kb == 2 * nkp - 1),
                                skip_group_check=True,
                            )
                            nc.tensor.matmul(
                                sum_ps[:],
                                ones_c[:],
                                pt2[:, i, :],
                                start=(kb == 0), stop=(kb == 2 * nkp - 1),
                                skip_group_check=True,
                            )
                        # dep-free PE filler: one c_proj group of the
                        # previous span per pair keeps the PE dense (and at
                        # full pstate) across the exp/mask sync bubbles
                        if pending_proj:
                            pending_proj.pop(0)()
                        st_cur = st_nxt
                    # drain ot to SBUF fast (frees the single ot bank), then
                    # normalize off the PE-critical path
                    otc = apool.tile([P, SPAN], bf16, tag="otc")
                    nc.scalar.copy(otc[:], ot_ps[:])
                    lns = apool.tile([1, SPAN], f32, tag="lns")
                    nc.scalar.activation(lns[:], sum_ps[:], AF.Ln)
                    rec = apool.tile([1, SPAN], f32r, tag="rec")
                    nc.scalar.activation(rec[:], lns[:], AF.Exp, scale=-1.0)
                    bc_ps = mm.tile([P, TCH], f32, tag="mm")
                    nc.tensor.matmul(bc_ps[:], ones_r[:], rec[:], start=True, stop=True)
                    nc.vector.tensor_mul(
                        ot_sb[:, h, q0 : q0 + SPAN], otc[:], bc_ps[:]
                    )

                # queue this span's output projection; it is emitted as PE
                # filler inside the next span's attention loop
                for tb in range(4 * s, 4 * s + 4):
                    for nch in range(C // 512):
                        pending_proj.append(proj_group(tb, nch))
            # last span's projection has no successor to hide in
            for fn in pending_proj:
                fn()
    nc.compile()
    n = _force_single_act_table(nc)
    print(f"act-table surgery: kept {n} table loads")
    return nc


_NC_CACHE = None


def _get_nc():
    global _NC_CACHE
    if _NC_CACHE is None:
        _NC_CACHE = build()
    return _NC_CACHE


def _host_inputs(x, cos, sin, wq, wk, wv, wo):
    """Build the 8 per-core input maps."""
    bft = ml_dtypes.bfloat16
    cosT = np.ascontiguousarray(cos[0, :, 0, :].T).astype(np.float32)  # (64, T)
    sinT = np.ascontiguousarray(sin[0, :, 0, :].T).astype(np.float32)
    cc = np.concatenate([cosT, cosT], axis=0).astype(bft)     # (128, T)
    ss = np.concatenate([sinT, -sinT], axis=0).astype(bft)
    # maskT[r][k, q] = 1 if q >= 128*r + k  (within a 512-q span, k-block offset r)
    qidx = np.arange(SPAN)[None, None, :]
    kidx = np.arange(P)[:, None, None]
    ridx = np.arange(4)[None, :, None]
    maskT = (qidx >= P * ridx + kidx).astype(bft)  # (128, 4, 512)

    xTs = [np.ascontiguousarray(x[b].T).astype(bft) for b in range(2)]
    wq16 = wq.astype(bft)
    wk16 = wk.astype(bft)
    wv16 = wv.astype(bft)
    wo16 = wo.astype(bft)
    in_maps = []
    for c in range(8):
        b, tp = divmod(c, 4)
        in_maps.append(
            {
                "xT": xTs[b],
                "wq": np.ascontiguousarray(wq16[:, tp * FQ : (tp + 1) * FQ]),
                "wk": np.ascontiguousarray(wk16[:, tp * FK : (tp + 1) * FK]),
                "wv": np.ascontiguousarray(wv16[:, tp * FK : (tp + 1) * FK]),
                "wo": np.ascontiguousarray(wo16[tp * FQ : (tp + 1) * FQ, :]),
                "cc": cc,
                "ss": ss,
                "maskT": maskT,
            }
        )
    return in_maps


def kernel(x, cos, sin, wq, wk, wv, wo, trace=False):
    x = np.asarray(x, dtype=np.float32)
    cos = np.asarray(cos, dtype=np.float32)
    sin = np.asarray(sin, dtype=np.float32)
    wq = np.asarray(wq, dtype=np.float32)
    wk = np.asarray(wk, dtype=np.float32)
    wv = np.asarray(wv, dtype=np.float32)
    wo = np.asarray(wo, dtype=np.float32)

    nc = _get_nc()
    in_maps = _host_inputs(x, cos, sin, wq, wk, wv, wo)
    res = run_bass_kernel_spmd(nc, in_maps, core_ids=list(range(8)), trace=trace)
    out = np.zeros((2, T, C), dtype=np.float32)
    for c in range(8):
        b = c // 4
        out[b] += np.asarray(res.results[c]["y"], dtype=np.float32)
    if trace:
        return out, res
    return out
